# revision 1
# baseline (speedup 1.0000x reference)
"""Trainium2 Bass kernel for nn_GameCraftVAEAttention.

Reference computation (B=2, S=4096, C=512, H=8 heads, D=64, GroupNorm G=32):
    x = group_norm(hidden_states)            # stats over (S, 16ch) per group
    q,k,v = x@wq+bq, x@wk+bk, x@wv+bv        # [B,S,512] -> heads [B,S,8,64]
    attn = softmax(q k^T / 8) v              # per (b,h)
    out = attn@wo + bo + hidden_states

Sharding: 16 (batch, head) pairs -> 8 cores, 2 heads (one batch) per core.
Core c: batch b=c//4, heads (2p, 2p+1) with p=c%4.  Each core computes
group-norm for its batch (replicated 4x, cheap), projections for its two
heads, attention, and a partial output projection partial^T = wo_h^T @ o^T.
Host unshard: out[b] = sum_partials^T + bo + residual.

On-core dataflow (everything transposed: channels/head-dim on partitions):
    x[b] --cast bf16--> scratch DRAM --DMA-transpose--> xbT [4x128, 4096]
    stats via DVE free-axis reduces + tiny selector matmuls -> per-channel
    scale/bias -> xnT bf16.  qT/kT/vT = w^T @ xnT (PE).  v transposed back
    via PE to [j, 64|ones] tiles.  Attention per s-chunk of 1024:
      scoresT[j-block, s] = kT^T-slice @ qT  (per head, PSUM)
      expT = Exp(scoresT) on ACT (no max subtraction: |scores| < ~2)
      o^T[65, s] += [v|1]^T @ expT  (PSUM accumulate over j; row 64 = rowsum)
    normalize by rowsum (DVE recip + K=1 ones-matmul broadcast), then
    partial^T = wo_slice^T @ o^T -> DMA out.
"""

import os
import sys

import numpy as np

sys.path.insert(0, "/opt/trn_rl_repo")

import concourse.bacc as bacc
import concourse.bass as bass
import concourse.mybir as mybir
import concourse.tile as tile
from concourse.bass_utils import run_bass_kernel_spmd

B, S, C = 2, 4096, 512
H, D = 8, 64
G = 32
EPS = 1e-6
N_CORES = 8
HPC = 2          # heads per core
D2 = HPC * D     # 128, stacked head dim
CP = 128         # channels per c-tile
NCT = C // CP    # 4 c-tiles
SCHUNK = 1024    # attention s-chunk
NSC = S // SCHUNK
JB = 128         # j block
NJB = S // JB
GPT = CP // (C // G)  # groups per c-tile = 8
CPG = C // G          # channels per group = 16

f32 = mybir.dt.float32
bf16 = mybir.dt.bfloat16
ts = bass.ts


def _body(ctx, tc):
    nc = tc.nc
    AF = mybir.ActivationFunctionType
    OP = mybir.AluOpType

    x_d = nc.dram_tensor("x", [S, C], f32, kind="ExternalInput").ap()
    wq_d = nc.dram_tensor("wq", [C, D2], f32, kind="ExternalInput").ap()
    wk_d = nc.dram_tensor("wk", [C, D2], f32, kind="ExternalInput").ap()
    wv_d = nc.dram_tensor("wv", [C, D2], f32, kind="ExternalInput").ap()
    wo_d = nc.dram_tensor("wo", [D2, C], f32, kind="ExternalInput").ap()
    bq_d = nc.dram_tensor("bq", [D2, 1], f32, kind="ExternalInput").ap()
    bk_d = nc.dram_tensor("bk", [D2, 1], f32, kind="ExternalInput").ap()
    bv_d = nc.dram_tensor("bv", [D2, 1], f32, kind="ExternalInput").ap()
    gnw_d = nc.dram_tensor("gnw", [C], f32, kind="ExternalInput").ap()
    gnb_d = nc.dram_tensor("gnb", [C], f32, kind="ExternalInput").ap()
    selg_d = nc.dram_tensor("selg", [CP, GPT], f32, kind="ExternalInput").ap()
    selb_d = nc.dram_tensor("selb", [GPT, CP], f32, kind="ExternalInput").ap()
    ident_d = nc.dram_tensor("ident", [D, D], bf16, kind="ExternalInput").ap()
    ones_d = nc.dram_tensor("ones", [1, D], f32, kind="ExternalInput").ap()
    pT_d = nc.dram_tensor("pT", [C, S], f32, kind="ExternalOutput").ap()
    xbf_d = nc.dram_tensor("xbf", [NCT, S, CP], bf16).ap()  # internal scratch

    # ---- persistent pools ----
    const_p = ctx.enter_context(tc.tile_pool(name="const", bufs=1))
    xbT_p = ctx.enter_context(tc.tile_pool(name="xbT", bufs=1))
    xnT_p = ctx.enter_context(tc.tile_pool(name="xnT", bufs=1))
    qkv_p = ctx.enter_context(tc.tile_pool(name="qkv", bufs=1))
    vaug_p = ctx.enter_context(tc.tile_pool(name="vaug", bufs=1))
    oT_p = ctx.enter_context(tc.tile_pool(name="oT", bufs=1))

    # ---- constants / weights into SBUF ----
    selg = const_p.tile([CP, GPT], f32)
    nc.sync.dma_start(selg[:], selg_d)
    selb = const_p.tile([GPT, CP], f32)
    nc.sync.dma_start(selb[:], selb_d)
    ident = const_p.tile([D, D], bf16)
    nc.sync.dma_start(ident[:], ident_d)
    ones = const_p.tile([1, D], f32)
    nc.sync.dma_start(ones[:], ones_d)

    w_sb = {}
    for name, wd in (("wq", wq_d), ("wk", wk_d), ("wv", wv_d)):
        t = const_p.tile([CP, NCT, D2], bf16, name=f"w_{name}", tag=f"w_{name}")
        nc.gpsimd.dma_start(t[:], wd.rearrange("(t p) d -> p t d", p=CP))
        w_sb[name] = t
    wo_sb = const_p.tile([D2, C], bf16)
    nc.gpsimd.dma_start(wo_sb[:], wo_d)
    b_sb = {}
    for name, bd in (("bq", bq_d), ("bk", bk_d)):
        t = const_p.tile([D2, 1], f32, name=f"b_{name}", tag=f"b_{name}")
        nc.sync.dma_start(t[:], bd)
        b_sb[name] = t
    bv_sb = const_p.tile([D, HPC], f32)
    nc.sync.dma_start(bv_sb[:], bv_d.rearrange("(h p) o -> p (h o)", p=D))
    gnw = const_p.tile([CP, NCT], f32)
    nc.sync.dma_start(gnw[:], gnw_d.rearrange("(t p) -> p t", p=CP))
    gnb = const_p.tile([CP, NCT], f32)
    nc.sync.dma_start(gnb[:], gnb_d.rearrange("(t p) -> p t", p=CP))

    # ---- phase A: x --cast+split--> bf16 scratch [NCT,S,CP] --transpose--> xbT ----
    xbT = [xbT_p.tile([CP, S], bf16, tag=f"xbT{t}", name=f"xbT{t}") for t in range(NCT)]
    with tc.tile_pool(name="xa", bufs=4) as xa_p:
        for st in range(S // CP):
            xb = xa_p.tile([CP, C], bf16)
            nc.gpsimd.dma_start(xb[:], x_d[st * CP : (st + 1) * CP, :])  # f32->bf16
            for ct in range(NCT):
                nc.sync.dma_start(
                    xbf_d[ct][st * CP : (st + 1) * CP, :], xb[:, ts(ct, CP)]
                )
    for t in range(NCT):
        nc.sync.dma_start(xbT[t][:], xbf_d[t], transpose=True)

    if os.environ.get("KERNEL_PHASES") == "A":
        for t in range(NCT):
            nc.gpsimd.dma_start(pT_d.rearrange("(a p) s -> a p s", p=CP)[t], xbT[t][:])
        return

    # ---- phase B/C/D: group-norm stats -> xnT ----
    xnT = [xnT_p.tile([CP, S], bf16, tag=f"xnT{t}", name=f"xnT{t}") for t in range(NCT)]
    with tc.tile_pool(name="gn_sc", bufs=2) as sq_p, \
         tc.tile_pool(name="gn_st", bufs=1) as st_p, \
         tc.tile_pool(name="gn_ps", bufs=2, space="PSUM") as gps_p:
        st = st_p.tile([CP, 2 * NCT], f32)
        for t in range(NCT):
            nc.vector.reduce_sum(st[:, t : t + 1], xbT[t][:], axis=mybir.AxisListType.X)
            sq = sq_p.tile([CP, S], f32)
            nc.vector.tensor_tensor(sq[:], xbT[t][:], xbT[t][:], op=OP.mult)
            nc.vector.reduce_sum(
                st[:, NCT + t : NCT + t + 1], sq[:], axis=mybir.AxisListType.X
            )
        gst_ps = gps_p.tile([GPT, 2 * NCT], f32)
        nc.tensor.matmul(gst_ps[:], lhsT=selg[:], rhs=st[:], start=True, stop=True)
        # tiny group-stat math on [8, NCT]
        gm = st_p.tile([GPT, 2 * NCT], f32)  # cols 0:4 mean, 4:8 rstd
        inv_n = 1.0 / (CPG * S)
        nc.vector.tensor_scalar_mul(gm[:, 0:NCT], gst_ps[:, 0:NCT], inv_n)
        ex2 = st_p.tile([GPT, NCT], f32)
        nc.vector.tensor_scalar_mul(ex2[:], gst_ps[:, NCT:], inv_n)
        var = st_p.tile([GPT, NCT], f32)
        nc.vector.tensor_tensor(var[:], gm[:, 0:NCT], gm[:, 0:NCT], op=OP.mult)
        nc.vector.tensor_tensor(var[:], ex2[:], var[:], op=OP.subtract)
        eps_t = st_p.tile([GPT, 1], f32)
        nc.vector.memset(eps_t[:], EPS)
        lnv = st_p.tile([GPT, NCT], f32)
        nc.scalar.activation(lnv[:], var[:], AF.Ln, bias=eps_t[:])
        nc.scalar.activation(gm[:, NCT:], lnv[:], AF.Exp, scale=-0.5)

        for t in range(NCT):
            bcm_ps = gps_p.tile([CP, 1], f32, tag="bc")
            nc.tensor.matmul(bcm_ps[:], lhsT=selb[:], rhs=gm[:, t : t + 1], start=True, stop=True)
            bcr_ps = gps_p.tile([CP, 1], f32, tag="bc")
            nc.tensor.matmul(bcr_ps[:], lhsT=selb[:], rhs=gm[:, NCT + t : NCT + t + 1], start=True, stop=True)
            scale_t = st_p.tile([CP, 1], f32, tag=f"sc{t}")
            nc.vector.tensor_tensor(scale_t[:], bcr_ps[:], gnw[:, t : t + 1], op=OP.mult)
            bias_t = st_p.tile([CP, 1], f32, tag=f"bi{t}")
            nc.vector.tensor_tensor(bias_t[:], bcm_ps[:], scale_t[:], op=OP.mult)
            nc.vector.tensor_tensor(bias_t[:], gnb[:, t : t + 1], bias_t[:], op=OP.subtract)
            nc.vector.tensor_scalar(
                xnT[t][:], xbT[t][:], scale_t[:], bias_t[:], op0=OP.mult, op1=OP.add
            )

    if os.environ.get("KERNEL_PHASES") == "D":
        for t in range(NCT):
            nc.gpsimd.dma_start(pT_d.rearrange("(a p) s -> a p s", p=CP)[t], xnT[t][:])
        return

    # ---- phase E: projections qT/kT/vT = w^T @ xnT  ([128, 4096] bf16) ----
    qT = qkv_p.tile([D2, S], bf16)
    kT = qkv_p.tile([D2, S], bf16)
    vTh = [qkv_p.tile([D, S], bf16, name=f"vTh{h}") for h in range(HPC)]
    with tc.tile_pool(name="proj_ps", bufs=3, space="PSUM") as pps:
        for wname, dst, bias, post in (
            ("wq", qT, b_sb["bq"], None),
            ("wk", kT, b_sb["bk"], 0.125),
        ):
            w = w_sb[wname]
            for n in range(S // 512):
                ps = pps.tile([D2, 512], f32)
                for ct in range(NCT):
                    nc.tensor.matmul(
                        ps[:],
                        lhsT=w[:, ct, :],
                        rhs=xnT[ct][:, ts(n, 512)],
                        start=(ct == 0),
                        stop=(ct == NCT - 1),
                    )
                if post is None:
                    nc.vector.tensor_scalar_add(dst[:, ts(n, 512)], ps[:], bias[:])
                else:
                    nc.vector.tensor_scalar(
                        dst[:, ts(n, 512)], ps[:], bias[:], post, op0=OP.add, op1=OP.mult
                    )
        # v: two per-head M=64 chains so vTh tiles sit at base partition 0
        wv = w_sb["wv"]
        for h in range(HPC):
            for n in range(S // 512):
                ps = pps.tile([D, 512], f32, tag="vps")
                for ct in range(NCT):
                    nc.tensor.matmul(
                        ps[:],
                        lhsT=wv[:, ct, h * D : (h + 1) * D],
                        rhs=xnT[ct][:, ts(n, 512)],
                        start=(ct == 0),
                        stop=(ct == NCT - 1),
                    )
                nc.vector.tensor_scalar_add(
                    vTh[h][:, ts(n, 512)], ps[:], bv_sb[:, h : h + 1]
                )

    # ---- phase F: vaug[j-tile] = [v_h0 | 1 | v_h1 | 1]  ([128, 130] bf16) ----
    vaug = [vaug_p.tile([JB, 2 * (D + 1)], bf16, tag=f"va{t}", name=f"va{t}") for t in range(NJB)]
    with tc.tile_pool(name="tp_ps", bufs=4, space="PSUM") as tps:
        for t in range(NJB):
            for h in range(HPC):
                tp = tps.tile([JB, D], bf16)
                nc.tensor.transpose(tp[:], vTh[h][:, ts(t, JB)], ident[:])
                nc.vector.tensor_copy(
                    vaug[t][:, h * (D + 1) : h * (D + 1) + D], tp[:]
                )
            nc.vector.memset(vaug[t][:, D : D + 1], 1.0)
            nc.vector.memset(vaug[t][:, 2 * D + 1 : 2 * D + 2], 1.0)

    if os.environ.get("KERNEL_PHASES") == "F":
        # debug bisect: dump qT/kT and first vaug tiles, skip attention/wo
        nc.gpsimd.dma_start(pT_d.rearrange("(a p) s -> a p s", p=CP)[0], qT[:])
        nc.gpsimd.dma_start(pT_d.rearrange("(a p) s -> a p s", p=CP)[1], kT[:])
        for t in range(8):
            nc.gpsimd.dma_start(
                pT_d.rearrange("(a p) s -> a p s", p=CP)[2][:, t * 130 : t * 130 + 130],
                vaug[t][:],
            )
        return

    # ---- phase G: attention ----
    oT = oT_p.tile([D2, S], bf16)
    with tc.tile_pool(name="sc_ps", bufs=2, space="PSUM") as sps, \
         tc.tile_pool(name="o_ps", bufs=1, space="PSUM") as ops, \
         tc.tile_pool(name="ex_sb", bufs=4) as exp_p, \
         tc.tile_pool(name="nrm_sb", bufs=4) as nrm_p:
        for sc in range(NSC):
            o_ps = [ops.tile([D + 1, SCHUNK], f32, tag=f"o{h}", name=f"ops_{sc}_{h}") for h in range(HPC)]
            for j in range(NJB):
                for h in range(HPC):
                    ps = sps.tile([JB, SCHUNK], f32)
                    for n2 in range(SCHUNK // 512):
                        nc.tensor.matmul(
                            ps[:, ts(n2, 512)],
                            lhsT=kT[h * D : (h + 1) * D, ts(j, JB)],
                            rhs=qT[h * D : (h + 1) * D, sc * SCHUNK + n2 * 512 : sc * SCHUNK + (n2 + 1) * 512],
                            start=True,
                            stop=True,
                        )
                    ex = exp_p.tile([JB, SCHUNK], bf16)
                    nc.scalar.activation(ex[:], ps[:], AF.Exp)
                    for n2 in range(SCHUNK // 512):
                        nc.tensor.matmul(
                            o_ps[h][:, ts(n2, 512)],
                            lhsT=vaug[j][:, h * (D + 1) : (h + 1) * (D + 1)],
                            rhs=ex[:, ts(n2, 512)],
                            start=(j == 0),
                            stop=(j == NJB - 1),
                        )
            for h in range(HPC):
                lnr = nrm_p.tile([1, SCHUNK], f32, tag="lnr")
                nc.scalar.activation(lnr[:], o_ps[h][D : D + 1, :], AF.Ln)
                rec = nrm_p.tile([1, SCHUNK], f32, tag="rec")
                nc.scalar.activation(rec[:], lnr[:], AF.Exp, scale=-1.0)
                bc = ops.tile([D, SCHUNK], f32, tag="o0", name=f"bc_{sc}_{h}")
                for n2 in range(SCHUNK // 512):
                    nc.tensor.matmul(
                        bc[:, ts(n2, 512)],
                        lhsT=ones[:],
                        rhs=rec[:, ts(n2, 512)],
                        start=True,
                        stop=True,
                    )
                o_f = nrm_p.tile([D, SCHUNK], f32, tag="of")
                nc.vector.tensor_copy(o_f[:], o_ps[h][0:D, :])
                nc.vector.tensor_tensor(
                    oT[h * D : (h + 1) * D, ts(sc, SCHUNK)], o_f[:], bc[:], op=OP.mult
                )

    # ---- phase H: partial^T = wo_slice^T @ oT -> DRAM ----
    pT_v = pT_d.rearrange("(t p) s -> t p s", p=CP)
    with tc.tile_pool(name="wo_ps", bufs=3, space="PSUM") as wps, \
         tc.tile_pool(name="wo_sb2", bufs=3) as wsb:
        for cc in range(NCT):
            for n in range(S // 512):
                ps = wps.tile([CP, 512], f32)
                nc.tensor.matmul(
                    ps[:],
                    lhsT=wo_sb[:, ts(cc, CP)],
                    rhs=oT[:, ts(n, 512)],
                    start=True,
                    stop=True,
                )
                ot = wsb.tile([CP, 512], f32)
                nc.vector.tensor_copy(ot[:], ps[:])
                nc.sync.dma_start(pT_v[cc][:, ts(n, 512)], ot[:])


_CACHE = {}


def _build():
    if "nc" in _CACHE:
        return _CACHE["nc"]
    import contextlib

    nc = bacc.Bacc("TRN2", target_bir_lowering=False, debug=False, enable_asserts=False)
    with tile.TileContext(nc) as tc:
        with contextlib.ExitStack() as ctx:
            _body(ctx, tc)
    nc.compile()
    _CACHE["nc"] = nc
    return nc


def _in_maps(inputs):
    x = np.ascontiguousarray(np.asarray(inputs["hidden_states"], dtype=np.float32))
    selg = (np.arange(CP)[:, None] // CPG == np.arange(GPT)[None, :]).astype(np.float32)
    selb = np.ascontiguousarray(selg.T)
    ident = np.eye(D, dtype=np.float32).astype(mybir.dt.np(bf16))
    ones = np.ones((1, D), dtype=np.float32)
    maps = []
    for c in range(N_CORES):
        b = c // (N_CORES // B)
        p = c % (N_CORES // B)
        sl = slice(p * D2, (p + 1) * D2)
        maps.append(
            {
                "x": x[b],
                "wq": np.ascontiguousarray(np.asarray(inputs["wq"], np.float32)[:, sl]),
                "wk": np.ascontiguousarray(np.asarray(inputs["wk"], np.float32)[:, sl]),
                "wv": np.ascontiguousarray(np.asarray(inputs["wv"], np.float32)[:, sl]),
                "wo": np.ascontiguousarray(np.asarray(inputs["wo"], np.float32)[sl, :]),
                "bq": np.ascontiguousarray(np.asarray(inputs["bq"], np.float32)[sl, None]),
                "bk": np.ascontiguousarray(np.asarray(inputs["bk"], np.float32)[sl, None]),
                "bv": np.ascontiguousarray(np.asarray(inputs["bv"], np.float32)[sl, None]),
                "gnw": np.asarray(inputs["gn_w"], np.float32),
                "gnb": np.asarray(inputs["gn_b"], np.float32),
                "selg": selg,
                "selb": selb,
                "ident": ident,
                "ones": ones,
            }
        )
    return maps


def _assemble(inputs, results):
    x = np.asarray(inputs["hidden_states"], dtype=np.float32)
    bo = np.asarray(inputs["bo"], dtype=np.float32)
    out = np.zeros((B, S, C), dtype=np.float32)
    for c in range(N_CORES):
        b = c // (N_CORES // B)
        out[b] += results[c]["pT"].T
    out += bo
    out += x
    return out


def kernel(**inputs):
    nc = _build()
    maps = _in_maps(inputs)
    res = run_bass_kernel_spmd(nc, maps, list(range(N_CORES)))
    return _assemble(inputs, res.results)


if __name__ == "__main__":
    nc = _build()
    print("built ok;", len(nc.m.functions[0].instructions) if hasattr(nc.m.functions[0], "instructions") else "")



# revision 12
# speedup vs baseline: 1.1599x; 1.1599x over previous
"""Trainium2 Bass kernel for nn_GameCraftVAEAttention.

Reference computation (B=2, S=4096, C=512, H=8 heads, D=64, GroupNorm G=32):
    x = group_norm(hidden_states)            # stats over (S, 16ch) per group
    q,k,v = x@wq+bq, x@wk+bk, x@wv+bv        # [B,S,512] -> heads [B,S,8,64]
    attn = softmax(q k^T / 8) v              # per (b,h)
    out = attn@wo + bo + hidden_states
Sharding: 16 (batch, head) pairs -> 8 cores, 2 heads (one batch) per core.
Host unshard: out[b] = sum_partials^T + bo + residual.

v2 on-core dataflow (vs v1: no DRAM scratch / DMA transpose, PE transposes
x while loading; stats on ACT+DVE per chunk; v projected directly into
[j, d] layout; recip on DVE; normalize on Pool; bf16 output partials):
  P1: 32x DMA x chunk [128s, 512c] f32 -> 4 PE transposes -> tp PSUM bf16
      -> Pool copy into xbT [128c, 4ct, 4096s]; ACT Square + DVE reduces
      accumulate per-chunk channel stats; group-stat math; xnT = scale*x+bias.
  P2: qT/kT = w^T @ xnT (PE, [128,4096] bf16); v direct: per j-block
      vps[j,d2] = xnT_slice^T @ wv + ones*bv -> vaug [128, 32, 130] (v|1).
  P3: per sc-chunk 1024, per j-block 128, per h: scoresT = kT_slice^T@qT
      (PSUM [128,1024]); exp on ACT -> bf16; oT[65,1024] += vaug^T @ exp
      (row 64 = rowsum); tail: DVE recip, Pool cast, PE ones-bcast (bf16),
      Pool normalize mult -> oT.
  P4 (interleaved per sc): pT = wo_slice^T @ oT -> Pool copy bf16 -> DMA.
"""

import os
import sys

import numpy as np

sys.path.insert(0, "/opt/trn_rl_repo")

import concourse.bacc as bacc
import concourse.bass as bass
import concourse.mybir as mybir
import concourse.tile as tile
from concourse.bass_utils import run_bass_kernel_spmd

B, S, C = 2, 4096, 512
H, D = 8, 64
G = 32
EPS = 1e-6
N_CORES = 8
HPC = 2          # heads per core
D2 = HPC * D     # 128, stacked head dim
CP = 128         # channels per c-tile
NCT = C // CP    # 4 c-tiles
SCHUNK = 1024    # attention s-chunk
NSC = S // SCHUNK
JB = 128         # j block
NJB = S // JB
NST = S // CP    # 32 s-chunks of 128
GPT = CP // (C // G)  # groups per c-tile = 8
CPG = C // G          # channels per group = 16

f32 = mybir.dt.float32
bf16 = mybir.dt.bfloat16
ts = bass.ts


def _body(ctx, tc):
    nc = tc.nc
    AF = mybir.ActivationFunctionType
    OP = mybir.AluOpType

    x_d = nc.dram_tensor("x", [S, C], f32, kind="ExternalInput").ap()
    wq_d = nc.dram_tensor("wq", [C, D2], f32, kind="ExternalInput").ap()
    wk_d = nc.dram_tensor("wk", [C, D2], f32, kind="ExternalInput").ap()
    wv_d = nc.dram_tensor("wv", [C, D2], f32, kind="ExternalInput").ap()
    wo_d = nc.dram_tensor("wo", [D2, C], f32, kind="ExternalInput").ap()
    bq_d = nc.dram_tensor("bq", [D2, 1], f32, kind="ExternalInput").ap()
    bk_d = nc.dram_tensor("bk", [D2, 1], f32, kind="ExternalInput").ap()
    bv_d = nc.dram_tensor("bv", [1, D2], f32, kind="ExternalInput").ap()
    gnw_d = nc.dram_tensor("gnw", [C], f32, kind="ExternalInput").ap()
    gnb_d = nc.dram_tensor("gnb", [C], f32, kind="ExternalInput").ap()
    selg_d = nc.dram_tensor("selg", [CP, GPT], f32, kind="ExternalInput").ap()
    selb_d = nc.dram_tensor("selb", [GPT, CP], f32, kind="ExternalInput").ap()
    id128_d = nc.dram_tensor("id128", [CP, CP], f32, kind="ExternalInput").ap()
    ones64_d = nc.dram_tensor("ones64", [1, D], bf16, kind="ExternalInput").ap()
    ones1_d = nc.dram_tensor("ones1", [1, CP], bf16, kind="ExternalInput").ap()
    pT_d = nc.dram_tensor("pT", [C, S], bf16, kind="ExternalOutput").ap()

    # ---- persistent pools ----
    const_p = ctx.enter_context(tc.tile_pool(name="const", bufs=1))
    xbT_p = ctx.enter_context(tc.tile_pool(name="xbT", bufs=1))
    qkv_p = ctx.enter_context(tc.tile_pool(name="qkv", bufs=1))
    oT_p = ctx.enter_context(tc.tile_pool(name="oT", bufs=1))

    # ---- constants / weights into SBUF ----
    selg = const_p.tile([CP, GPT], f32)
    nc.sync.dma_start(selg[:], selg_d)
    selb = const_p.tile([GPT, CP], f32)
    nc.sync.dma_start(selb[:], selb_d)
    id128 = const_p.tile([CP, CP], f32)
    nc.sync.dma_start(id128[:], id128_d)
    ones64 = const_p.tile([1, D], bf16)
    nc.sync.dma_start(ones64[:], ones64_d)
    ones1 = const_p.tile([1, CP], bf16)
    nc.sync.dma_start(ones1[:], ones1_d)
    bv_sb = const_p.tile([1, D2], bf16)
    nc.gpsimd.dma_start(bv_sb[:], bv_d)  # f32 -> bf16 cast on SWDGE

    w_sb = {}
    for name, wd in (("wq", wq_d), ("wk", wk_d), ("wv", wv_d)):
        t = const_p.tile([CP, NCT, D2], bf16, name=f"w_{name}", tag=f"w_{name}")
        nc.gpsimd.dma_start(t[:], wd.rearrange("(t p) d -> p t d", p=CP))
        w_sb[name] = t
    wo_sb = const_p.tile([D2, C], bf16)
    nc.gpsimd.dma_start(wo_sb[:], wo_d)
    b_sb = {}
    for name, bd in (("bq", bq_d), ("bk", bk_d)):
        t = const_p.tile([D2, 1], f32, name=f"b_{name}", tag=f"b_{name}")
        nc.sync.dma_start(t[:], bd)
        b_sb[name] = t
    gnw = const_p.tile([CP, NCT], f32)
    nc.sync.dma_start(gnw[:], gnw_d.rearrange("(t p) -> p t", p=CP))
    gnb = const_p.tile([CP, NCT], f32)
    nc.sync.dma_start(gnb[:], gnb_d.rearrange("(t p) -> p t", p=CP))

    # ---- P1: load x chunks, PE-transpose into xbT, per-chunk stats ----
    # xbT[:, ct, s] == x[s, ct*128 + p] cast to bf16
    xbT = xbT_p.tile([CP, NCT, S], bf16)
    xnT = xbT_p.tile([CP, NCT, S], bf16)
    stx = xbT_p.tile([CP, NST, NCT], bf16, name="stx")
    stx2 = xbT_p.tile([CP, NST, NCT], bf16, name="stx2")
    scale_t = [None] * NCT
    bias_t = [None] * NCT
    with tc.tile_pool(name="xa", bufs=4) as xa_p, \
         tc.tile_pool(name="tp_ps", bufs=3, space="PSUM") as tp_p, \
         tc.tile_pool(name="dead", bufs=2) as dead_p, \
         tc.tile_pool(name="gn_st", bufs=1) as st_p, \
         tc.tile_pool(name="gn_ps", bufs=2, space="PSUM") as gps_p:
        for st in range(NST):
            xb = xa_p.tile([CP, C], f32)
            nc.sync.dma_start(xb[:], x_d[st * CP : (st + 1) * CP, :])
            tp = tp_p.tile([CP, C], f32)
            for ct in range(NCT):
                nc.tensor.transpose(
                    tp[:, ts(ct, CP)], xb[:, ts(ct, CP)], id128[:]
                )
            tpv = tp[:].rearrange("p (a b) -> p a b", a=NCT)
            xslice = xbT[:, :, st * CP : (st + 1) * CP]
            nc.scalar.activation(xslice, tpv, AF.Copy)  # PSUM f32 -> SBUF bf16
            with nc.allow_low_precision(reason="bf16 chunk stat partials; final sum in f32"):
                nc.vector.reduce_sum(
                    stx[:, st, :], xslice, axis=mybir.AxisListType.X
                )
                sq = dead_p.tile([CP, C], bf16)
                sqv = sq[:].rearrange("p (a b) -> p a b", a=NCT)
                nc.vector.tensor_tensor(sqv, xslice, xslice, op=OP.mult)
                nc.vector.reduce_sum(
                    stx2[:, st, :], sqv, axis=mybir.AxisListType.X
                )

        # ---- group-norm stats -> scale/bias per c-tile ----
        st_all = st_p.tile([CP, 2 * NCT], f32)
        nc.vector.reduce_sum(
            st_all[:, 0:NCT],
            stx[:].rearrange("p a b -> p b a"),
            axis=mybir.AxisListType.X,
        )
        nc.vector.reduce_sum(
            st_all[:, NCT:],
            stx2[:].rearrange("p a b -> p b a"),
            axis=mybir.AxisListType.X,
        )
        gst_ps = gps_p.tile([GPT, 2 * NCT], f32)
        nc.tensor.matmul(gst_ps[:], lhsT=selg[:], rhs=st_all[:], start=True, stop=True)
        gm = st_p.tile([GPT, 2 * NCT], f32)  # cols 0:4 mean, 4:8 rstd
        inv_n = 1.0 / (CPG * S)
        nc.vector.tensor_scalar_mul(gm[:, 0:NCT], gst_ps[:, 0:NCT], inv_n)
        ex2 = st_p.tile([GPT, NCT], f32)
        nc.vector.tensor_scalar_mul(ex2[:], gst_ps[:, NCT:], inv_n)
        var = st_p.tile([GPT, NCT], f32)
        nc.vector.tensor_tensor(var[:], gm[:, 0:NCT], gm[:, 0:NCT], op=OP.mult)
        nc.vector.tensor_tensor(var[:], ex2[:], var[:], op=OP.subtract)
        eps_t = st_p.tile([GPT, 1], f32)
        nc.vector.memset(eps_t[:], EPS)
        lnv = st_p.tile([GPT, NCT], f32)
        nc.scalar.activation(lnv[:], var[:], AF.Ln, bias=eps_t[:])
        nc.scalar.activation(gm[:, NCT:], lnv[:], AF.Exp, scale=-0.5)

        for t in range(NCT):
            bcm_ps = gps_p.tile([CP, 1], f32, tag="bc")
            nc.tensor.matmul(bcm_ps[:], lhsT=selb[:], rhs=gm[:, t : t + 1], start=True, stop=True)
            bcr_ps = gps_p.tile([CP, 1], f32, tag="bc")
            nc.tensor.matmul(bcr_ps[:], lhsT=selb[:], rhs=gm[:, NCT + t : NCT + t + 1], start=True, stop=True)
            sc_t = st_p.tile([CP, 1], f32, tag=f"sc{t}")
            nc.vector.tensor_tensor(sc_t[:], bcr_ps[:], gnw[:, t : t + 1], op=OP.mult)
            bi_t = st_p.tile([CP, 1], f32, tag=f"bi{t}")
            nc.vector.tensor_tensor(bi_t[:], bcm_ps[:], sc_t[:], op=OP.mult)
            nc.vector.tensor_tensor(bi_t[:], gnb[:, t : t + 1], bi_t[:], op=OP.subtract)
            scale_t[t] = sc_t
            bias_t[t] = bi_t
            eng = nc.vector if t % 2 == 0 else nc.gpsimd
            eng.tensor_scalar(
                xnT[:, t, :], xbT[:, t, :], sc_t[:], bi_t[:], op0=OP.mult, op1=OP.add
            )

    if os.environ.get("KERNEL_PHASES") == "D":
        for t in range(NCT):
            nc.gpsimd.dma_start(pT_d.rearrange("(a p) s -> a p s", p=CP)[t], xnT[:, t, :])
        return

    # ---- P2: projections ----
    qT = qkv_p.tile([D2, S], bf16)
    kT = qkv_p.tile([D2, S], bf16)
    vaug = qkv_p.tile([CP, NJB, HPC * (D + 1)], bf16, name="vaug")
    nc.gpsimd.memset(vaug[:], 1.0)  # ones cols (64, 129) survive the copies
    with tc.tile_pool(name="proj_ps", bufs=3, space="PSUM") as pps, \
         tc.tile_pool(name="vp_ps", bufs=3, space="PSUM") as vps_p:
        for wname, dst, bias, post in (
            ("wq", qT, b_sb["bq"], None),
            ("wk", kT, b_sb["bk"], 0.125),
        ):
            w = w_sb[wname]
            for n in range(S // 512):
                ps = pps.tile([D2, 512], f32)
                for ct in range(NCT):
                    nc.tensor.matmul(
                        ps[:],
                        lhsT=w[:, ct, :],
                        rhs=xnT[:, ct, ts(n, 512)],
                        start=(ct == 0),
                        stop=(ct == NCT - 1),
                    )
                if post is None:
                    nc.vector.tensor_scalar_add(dst[:, ts(n, 512)], ps[:], bias[:])
                else:
                    nc.vector.tensor_scalar(
                        dst[:, ts(n, 512)], ps[:], bias[:], post, op0=OP.add, op1=OP.mult
                    )
        # v directly in [j, d2] layout: vps = xnT_slice^T @ wv + ones x bv
        wv = w_sb["wv"]
        for j in range(NJB):
            vps = vps_p.tile([CP, D2], f32)
            for ct in range(NCT):
                nc.tensor.matmul(
                    vps[:],
                    lhsT=xnT[:, ct, ts(j, JB)],
                    rhs=wv[:, ct, :],
                    start=(ct == 0),
                    stop=False,
                )
            nc.tensor.matmul(
                vps[:], lhsT=ones1[:, 0:JB], rhs=bv_sb[:], start=False, stop=True
            )
            # scatter [j, (h d)] -> vaug[:, j, h*(D+1) : h*(D+1)+D]
            nc.scalar.activation(
                vaug[:, j, :].rearrange("p (h e) -> p h e", h=HPC)[:, :, 0:D],
                vps[:].rearrange("p (h d) -> p h d", h=HPC),
                AF.Copy,
            )

    if os.environ.get("KERNEL_PHASES") == "F":
        nc.gpsimd.dma_start(pT_d.rearrange("(a p) s -> a p s", p=CP)[0], qT[:])
        nc.gpsimd.dma_start(pT_d.rearrange("(a p) s -> a p s", p=CP)[1], kT[:])
        for t in range(8):
            nc.gpsimd.dma_start(
                pT_d.rearrange("(a p) s -> a p s", p=CP)[2][:, t * 130 : t * 130 + 130],
                vaug[:, t, :],
            )
        return

    # ---- P3: attention + interleaved P4 (wo projection of finished chunks) ----
    oT = oT_p.tile([D2, S], bf16)
    pT_v = pT_d.rearrange("(t p) s -> t p s", p=CP)
    with tc.tile_pool(name="sc_ps", bufs=2, space="PSUM") as sps, \
         tc.tile_pool(name="o_ps", bufs=1, space="PSUM") as ops, \
         tc.tile_pool(name="ex_sb", bufs=4) as exp_p, \
         tc.tile_pool(name="nrm_sb", bufs=4) as nrm_p, \
         tc.tile_pool(name="wo_sb2", bufs=3) as wsb:

        def wo_chunk(sc):
            # partial^T for s-chunk sc: 4 cc x 2 n2 matmuls from oT
            for cc in range(NCT):
                for n2 in range(SCHUNK // 512):
                    wps = ops.tile([CP, 512], f32, tag=f"o{cc % 2}",
                                   name=f"wops_{sc}_{cc}_{n2}")
                    nc.tensor.matmul(
                        wps[:],
                        lhsT=wo_sb[:, ts(cc, CP)],
                        rhs=oT[:, sc * SCHUNK + n2 * 512 : sc * SCHUNK + (n2 + 1) * 512],
                        start=True,
                        stop=True,
                    )
                    ot = wsb.tile([CP, 512], bf16)
                    nc.vector.tensor_copy(ot[:], wps[:])
                    nc.sync.dma_start(
                        pT_v[cc][:, sc * SCHUNK + n2 * 512 : sc * SCHUNK + (n2 + 1) * 512],
                        ot[:],
                    )

        for sc in range(NSC):
            o_ps = [
                ops.tile([D + 1, SCHUNK], f32, tag=f"o{h}", name=f"ops_{sc}_{h}")
                for h in range(HPC)
            ]
            for j in range(NJB):
                for h in range(HPC):
                    ps = sps.tile([JB, SCHUNK], f32)
                    for n2 in range(SCHUNK // 512):
                        nc.tensor.matmul(
                            ps[:, ts(n2, 512)],
                            lhsT=kT[h * D : (h + 1) * D, ts(j, JB)],
                            rhs=qT[h * D : (h + 1) * D, sc * SCHUNK + n2 * 512 : sc * SCHUNK + (n2 + 1) * 512],
                            start=True,
                            stop=True,
                        )
                    ex = exp_p.tile([JB, SCHUNK], bf16)
                    nc.scalar.activation(ex[:], ps[:], AF.Exp)
                    for n2 in range(SCHUNK // 512):
                        nc.tensor.matmul(
                            o_ps[h][:, ts(n2, 512)],
                            lhsT=vaug[:, j, h * (D + 1) : (h + 1) * (D + 1)],
                            rhs=ex[:, ts(n2, 512)],
                            start=(j == 0),
                            stop=(j == NJB - 1),
                        )
            for h in range(HPC):
                rec = nrm_p.tile([1, SCHUNK], f32, tag="rec")
                nc.vector.reciprocal(rec[:], o_ps[h][D : D + 1, :])
                recb = nrm_p.tile([1, SCHUNK], bf16, tag="recb")
                nc.gpsimd.tensor_copy(recb[:], rec[:])
                o_f = nrm_p.tile([D, SCHUNK], f32, tag="of")
                nc.vector.tensor_copy(o_f[:], o_ps[h][0:D, :])
                bc = ops.tile([D, SCHUNK], f32, tag=f"o{h}", name=f"bc_{sc}_{h}")
                for n2 in range(SCHUNK // 512):
                    nc.tensor.matmul(
                        bc[:, ts(n2, 512)],
                        lhsT=ones64[:],
                        rhs=recb[:, ts(n2, 512)],
                        start=True,
                        stop=True,
                    )
                nc.vector.tensor_tensor(
                    oT[h * D : (h + 1) * D, ts(sc, SCHUNK)], o_f[:], bc[:], op=OP.mult
                )
            wo_chunk(sc)


_CACHE = {}


def _build():
    if "nc" in _CACHE:
        return _CACHE["nc"]
    import contextlib

    nc = bacc.Bacc("TRN2", target_bir_lowering=False, debug=False, enable_asserts=False)
    with tile.TileContext(nc) as tc:
        with contextlib.ExitStack() as ctx:
            _body(ctx, tc)
    nc.compile()
    _CACHE["nc"] = nc
    return nc


def _in_maps(inputs):
    x = np.ascontiguousarray(np.asarray(inputs["hidden_states"], dtype=np.float32))
    selg = (np.arange(CP)[:, None] // CPG == np.arange(GPT)[None, :]).astype(np.float32)
    selb = np.ascontiguousarray(selg.T)
    id128 = np.eye(CP, dtype=np.float32)
    bf = mybir.dt.np(bf16)
    ones64 = np.ones((1, D), dtype=np.float32).astype(bf)
    ones1 = np.ones((1, CP), dtype=np.float32).astype(bf)
    maps = []
    for c in range(N_CORES):
        b = c // (N_CORES // B)
        p = c % (N_CORES // B)
        sl = slice(p * D2, (p + 1) * D2)
        maps.append(
            {
                "x": x[b],
                "wq": np.ascontiguousarray(np.asarray(inputs["wq"], np.float32)[:, sl]),
                "wk": np.ascontiguousarray(np.asarray(inputs["wk"], np.float32)[:, sl]),
                "wv": np.ascontiguousarray(np.asarray(inputs["wv"], np.float32)[:, sl]),
                "wo": np.ascontiguousarray(np.asarray(inputs["wo"], np.float32)[sl, :]),
                "bq": np.ascontiguousarray(np.asarray(inputs["bq"], np.float32)[sl, None]),
                "bk": np.ascontiguousarray(np.asarray(inputs["bk"], np.float32)[sl, None]),
                "bv": np.ascontiguousarray(np.asarray(inputs["bv"], np.float32)[None, sl]),
                "gnw": np.asarray(inputs["gn_w"], np.float32),
                "gnb": np.asarray(inputs["gn_b"], np.float32),
                "selg": selg,
                "selb": selb,
                "id128": id128,
                "ones64": ones64,
                "ones1": ones1,
            }
        )
    return maps


def _assemble(inputs, results):
    x = np.asarray(inputs["hidden_states"], dtype=np.float32)
    bo = np.asarray(inputs["bo"], dtype=np.float32)
    out = np.zeros((B, S, C), dtype=np.float32)
    for c in range(N_CORES):
        b = c // (N_CORES // B)
        out[b] += results[c]["pT"].astype(np.float32).T
    out += bo
    out += x
    return out


def kernel(**inputs):
    nc = _build()
    maps = _in_maps(inputs)
    res = run_bass_kernel_spmd(nc, maps, list(range(N_CORES)))
    return _assemble(inputs, res.results)


if __name__ == "__main__":
    nc = _build()
    print("built ok")


# revision 26
# speedup vs baseline: 1.1908x; 1.0267x over previous
"""Trainium2 Bass kernel for nn_GameCraftVAEAttention.

Reference computation (B=2, S=4096, C=512, H=8 heads, D=64, GroupNorm G=32):
    x = group_norm(hidden_states)            # stats over (S, 16ch) per group
    q,k,v = x@wq+bq, x@wk+bk, x@wv+bv        # [B,S,512] -> heads [B,S,8,64]
    attn = softmax(q k^T / 8) v              # per (b,h)
    out = attn@wo + bo + hidden_states
Sharding: 16 (batch, head) pairs -> 8 cores, 2 heads (one batch) per core.
Host unshard: out[b] = sum_partials^T + bo + residual.

v2 on-core dataflow (vs v1: no DRAM scratch / DMA transpose, PE transposes
x while loading; stats on ACT+DVE per chunk; v projected directly into
[j, d] layout; recip on DVE; normalize on Pool; bf16 output partials):
  P1: 32x DMA x chunk [128s, 512c] f32 -> 4 PE transposes -> tp PSUM bf16
      -> Pool copy into xbT [128c, 4ct, 4096s]; ACT Square + DVE reduces
      accumulate per-chunk channel stats; group-stat math; xnT = scale*x+bias.
  P2: qT/kT = w^T @ xnT (PE, [128,4096] bf16); v direct: per j-block
      vps[j,d2] = xnT_slice^T @ wv + ones*bv -> vaug [128, 32, 130] (v|1).
  P3: per sc-chunk 1024, per j-block 128, per h: scoresT = kT_slice^T@qT
      (PSUM [128,1024]); exp on ACT -> bf16; oT[65,1024] += vaug^T @ exp
      (row 64 = rowsum); tail: DVE recip, Pool cast, PE ones-bcast (bf16),
      Pool normalize mult -> oT.
  P4 (interleaved per sc): pT = wo_slice^T @ oT -> Pool copy bf16 -> DMA.
"""

import os
import sys

import numpy as np

sys.path.insert(0, "/opt/trn_rl_repo")

import concourse.bacc as bacc
import concourse.bass as bass
import concourse.mybir as mybir
import concourse.tile as tile
from concourse.bass_utils import run_bass_kernel_spmd

B, S, C = 2, 4096, 512
H, D = 8, 64
G = 32
EPS = 1e-6
N_CORES = 8
HPC = 2          # heads per core
D2 = HPC * D     # 128, stacked head dim
CP = 128         # channels per c-tile
NCT = C // CP    # 4 c-tiles
SCHUNK = 1024    # attention s-chunk
NSC = S // SCHUNK
JB = 128         # j block
NJB = S // JB
NST = S // CP    # 32 s-chunks of 128
GPT = CP // (C // G)  # groups per c-tile = 8
CPG = C // G          # channels per group = 16

f32 = mybir.dt.float32
bf16 = mybir.dt.bfloat16
ts = bass.ts


def _body(ctx, tc):
    nc = tc.nc
    AF = mybir.ActivationFunctionType
    OP = mybir.AluOpType

    x_d = nc.dram_tensor("x", [S, C], f32, kind="ExternalInput").ap()
    wq_d = nc.dram_tensor("wq", [C, D2], f32, kind="ExternalInput").ap()
    wk_d = nc.dram_tensor("wk", [C, D2], f32, kind="ExternalInput").ap()
    wv_d = nc.dram_tensor("wv", [C, D2], f32, kind="ExternalInput").ap()
    wo_d = nc.dram_tensor("wo", [D2, C], f32, kind="ExternalInput").ap()
    bq_d = nc.dram_tensor("bq", [D2, 1], f32, kind="ExternalInput").ap()
    bk_d = nc.dram_tensor("bk", [D2, 1], f32, kind="ExternalInput").ap()
    bv_d = nc.dram_tensor("bv", [1, D2], f32, kind="ExternalInput").ap()
    gnw_d = nc.dram_tensor("gnw", [C], f32, kind="ExternalInput").ap()
    gnb_d = nc.dram_tensor("gnb", [C], f32, kind="ExternalInput").ap()
    selg_d = nc.dram_tensor("selg", [CP, GPT], f32, kind="ExternalInput").ap()
    selb_d = nc.dram_tensor("selb", [GPT, CP], f32, kind="ExternalInput").ap()
    id128_d = nc.dram_tensor("id128", [CP, CP], f32, kind="ExternalInput").ap()
    ones64_d = nc.dram_tensor("ones64", [1, D], f32, kind="ExternalInput").ap()
    ones1_d = nc.dram_tensor("ones1", [1, CP], bf16, kind="ExternalInput").ap()
    pT_d = nc.dram_tensor("pT", [C, S], bf16, kind="ExternalOutput").ap()

    # ---- persistent pools ----
    const_p = ctx.enter_context(tc.tile_pool(name="const", bufs=1))
    xbT_p = ctx.enter_context(tc.tile_pool(name="xbT", bufs=1))
    qkv_p = ctx.enter_context(tc.tile_pool(name="qkv", bufs=1))
    oT_p = ctx.enter_context(tc.tile_pool(name="oT", bufs=1))

    # ---- constants / weights into SBUF ----
    selg = const_p.tile([CP, GPT], f32)
    nc.sync.dma_start(selg[:], selg_d)
    selb = const_p.tile([GPT, CP], f32)
    nc.sync.dma_start(selb[:], selb_d)
    id128 = const_p.tile([CP, CP], f32)
    nc.sync.dma_start(id128[:], id128_d)
    ones64 = const_p.tile([1, D], f32)
    nc.sync.dma_start(ones64[:], ones64_d)
    ones1 = const_p.tile([1, CP], bf16)
    nc.sync.dma_start(ones1[:], ones1_d)
    bv_sb = const_p.tile([1, D2], bf16)
    nc.gpsimd.dma_start(bv_sb[:], bv_d)  # f32 -> bf16 cast on SWDGE

    w_sb = {}
    for name, wd in (("wq", wq_d), ("wk", wk_d), ("wv", wv_d)):
        t = const_p.tile([CP, NCT, D2], bf16, name=f"w_{name}", tag=f"w_{name}")
        nc.gpsimd.dma_start(t[:], wd.rearrange("(t p) d -> p t d", p=CP))
        w_sb[name] = t
    wo_sb = const_p.tile([D2, C], bf16)
    nc.gpsimd.dma_start(wo_sb[:], wo_d)
    b_sb = {}
    for name, bd in (("bq", bq_d), ("bk", bk_d)):
        t = const_p.tile([D2, 1], f32, name=f"b_{name}", tag=f"b_{name}")
        nc.sync.dma_start(t[:], bd)
        b_sb[name] = t
    gnw = const_p.tile([CP, NCT], f32)
    nc.sync.dma_start(gnw[:], gnw_d.rearrange("(t p) -> p t", p=CP))
    gnb = const_p.tile([CP, NCT], f32)
    nc.sync.dma_start(gnb[:], gnb_d.rearrange("(t p) -> p t", p=CP))

    # ---- P1: load x chunks, PE-transpose into xbT, per-chunk stats ----
    # xbT[:, ct, s] == x[s, ct*128 + p] cast to bf16
    xbT = xbT_p.tile([CP, NCT, S], bf16)
    stx = xbT_p.tile([CP, NST, NCT], bf16, name="stx")
    stx2 = xbT_p.tile([CP, NST, NCT], bf16, name="stx2")
    scale_t = [None] * NCT
    bias_t = [None] * NCT
    w2_sb = {}
    for name in ("wq", "wk", "wv"):
        w2_sb[name] = const_p.tile(
            [CP, NCT, D2], bf16, name=f"w2_{name}", tag=f"w2_{name}"
        )
    b2 = {
        "wq": const_p.tile([D2, 1], f32, name="b2_wq", tag="b2_wq"),
        "wk": const_p.tile([D2, 1], f32, name="b2_wk", tag="b2_wk"),
    }
    bv2 = const_p.tile([1, D2], bf16, name="bv2")
    with tc.tile_pool(name="xa", bufs=4) as xa_p, \
         tc.tile_pool(name="tp_ps", bufs=3, space="PSUM") as tp_p, \
         tc.tile_pool(name="dead", bufs=2) as dead_p, \
         tc.tile_pool(name="gn_st", bufs=1) as st_p, \
         tc.tile_pool(name="gn_ps", bufs=1, space="PSUM") as gps_p:
        for st in range(NST):
            xb = xa_p.tile([CP, C], f32)
            nc.sync.dma_start(xb[:], x_d[st * CP : (st + 1) * CP, :])
            tp = tp_p.tile([CP, C], f32)
            for ct in range(NCT):
                nc.tensor.transpose(
                    tp[:, ts(ct, CP)], xb[:, ts(ct, CP)], id128[:]
                )
            tpv = tp[:].rearrange("p (a b) -> p a b", a=NCT)
            xslice = xbT[:, :, st * CP : (st + 1) * CP]
            nc.scalar.activation(xslice, tpv, AF.Copy)  # PSUM f32 -> SBUF bf16
            with nc.allow_low_precision(reason="bf16 chunk stat partials; final sum in f32"):
                nc.vector.reduce_sum(
                    stx[:, st, :], xslice, axis=mybir.AxisListType.X
                )
                sq = dead_p.tile([CP, C], bf16)
                sqv = sq[:].rearrange("p (a b) -> p a b", a=NCT)
                nc.vector.tensor_tensor(sqv, xslice, xslice, op=OP.mult)
                nc.vector.reduce_sum(
                    stx2[:, st, :], sqv, axis=mybir.AxisListType.X
                )

        # ---- group-norm stats -> scale/bias per c-tile ----
        st_all = st_p.tile([CP, 2 * NCT], f32)
        nc.vector.reduce_sum(
            st_all[:, 0:NCT],
            stx[:].rearrange("p a b -> p b a"),
            axis=mybir.AxisListType.X,
        )
        nc.vector.reduce_sum(
            st_all[:, NCT:],
            stx2[:].rearrange("p a b -> p b a"),
            axis=mybir.AxisListType.X,
        )
        gst_ps = gps_p.tile([GPT, 2 * NCT], f32)
        nc.tensor.matmul(gst_ps[:], lhsT=selg[:], rhs=st_all[:], start=True, stop=True)
        gm = st_p.tile([GPT, 2 * NCT], f32)  # cols 0:4 mean, 4:8 rstd
        inv_n = 1.0 / (CPG * S)
        nc.vector.tensor_scalar_mul(gm[:, 0:NCT], gst_ps[:, 0:NCT], inv_n)
        ex2 = st_p.tile([GPT, NCT], f32)
        nc.vector.tensor_scalar_mul(ex2[:], gst_ps[:, NCT:], inv_n)
        var = st_p.tile([GPT, NCT], f32)
        nc.vector.tensor_tensor(var[:], gm[:, 0:NCT], gm[:, 0:NCT], op=OP.mult)
        nc.vector.tensor_tensor(var[:], ex2[:], var[:], op=OP.subtract)
        eps_t = st_p.tile([GPT, 1], f32)
        nc.vector.memset(eps_t[:], EPS)
        lnv = st_p.tile([GPT, NCT], f32)
        nc.scalar.activation(lnv[:], var[:], AF.Ln, bias=eps_t[:])
        nc.scalar.activation(gm[:, NCT:], lnv[:], AF.Exp, scale=-0.5)

        # fold groupnorm into the projection weights: w2 = scale * w,
        # b2 = b + w^T bias (q = (x*s+b) @ w + bq = x @ w2 + (bq + w^T b))
        bias_b = st_p.tile([CP, NCT], bf16, name="bias_b")
        for t in range(NCT):
            bcm_ps = gps_p.tile([CP, 1], f32, tag="bcm")
            nc.tensor.matmul(bcm_ps[:], lhsT=selb[:], rhs=gm[:, t : t + 1], start=True, stop=True)
            bcr_ps = gps_p.tile([CP, 1], f32, tag="bcr")
            nc.tensor.matmul(bcr_ps[:], lhsT=selb[:], rhs=gm[:, NCT + t : NCT + t + 1], start=True, stop=True)
            sc_t = st_p.tile([CP, 1], f32, tag=f"sc{t}")
            nc.vector.tensor_tensor(sc_t[:], bcr_ps[:], gnw[:, t : t + 1], op=OP.mult)
            bi_t = st_p.tile([CP, 1], f32, tag=f"bi{t}")
            nc.vector.tensor_tensor(bi_t[:], bcm_ps[:], sc_t[:], op=OP.mult)
            nc.vector.tensor_tensor(bi_t[:], gnb[:, t : t + 1], bi_t[:], op=OP.subtract)
            scale_t[t] = sc_t
            bias_t[t] = bi_t
            nc.vector.tensor_copy(bias_b[:, t : t + 1], bi_t[:])
            for wi, wname in enumerate(("wq", "wk", "wv")):
                eng = nc.vector if (t * 3 + wi) % 2 == 0 else nc.gpsimd
                eng.tensor_scalar_mul(
                    w2_sb[wname][:, t, :], w_sb[wname][:, t, :], sc_t[:]
                )
        # folded biases
        for wname, bsrc in (("wq", b_sb["bq"]), ("wk", b_sb["bk"])):
            ps = gps_p.tile([D2, 1], f32, tag="bcm")
            for ct in range(NCT):
                nc.tensor.matmul(
                    ps[:], lhsT=w_sb[wname][:, ct, :], rhs=bias_b[:, ct : ct + 1],
                    start=(ct == 0), stop=(ct == NCT - 1),
                )
            nc.vector.tensor_tensor(b2[wname][:], bsrc[:], ps[:], op=OP.add)
        psv = gps_p.tile([1, D2], f32, tag="bv")
        for ct in range(NCT):
            nc.tensor.matmul(
                psv[:], lhsT=bias_b[:, ct : ct + 1], rhs=w_sb["wv"][:, ct, :],
                start=(ct == 0), stop=(ct == NCT - 1),
            )
        nc.vector.tensor_tensor(bv2[:], psv[:], bv_sb[:], op=OP.add)

    # ---- P2: projections ----
    qT = qkv_p.tile([D2, S], bf16)
    kT = qkv_p.tile([D2, S], bf16)
    vaug = qkv_p.tile([CP, NJB, HPC * (D + 1)], bf16, name="vaug")
    nc.gpsimd.memset(vaug[:], 1.0)  # ones cols (64, 129) survive the copies
    with tc.tile_pool(name="proj_ps", bufs=3, space="PSUM") as pps, \
         tc.tile_pool(name="vp_ps", bufs=3, space="PSUM") as vps_p:
        for wname, dst, bias, post in (
            ("wq", qT, b2["wq"], None),
            ("wk", kT, b2["wk"], 0.125),
        ):
            w = w2_sb[wname]
            for n in range(S // 512):
                ps = pps.tile([D2, 512], f32)
                for ct in range(NCT):
                    nc.tensor.matmul(
                        ps[:],
                        lhsT=w[:, ct, :],
                        rhs=xbT[:, ct, ts(n, 512)],
                        start=(ct == 0),
                        stop=(ct == NCT - 1),
                    )
                if post is None:
                    nc.vector.tensor_scalar_add(dst[:, ts(n, 512)], ps[:], bias[:])
                else:
                    nc.vector.tensor_scalar(
                        dst[:, ts(n, 512)], ps[:], bias[:], post, op0=OP.add, op1=OP.mult
                    )
        # v directly in [j, d2] layout: vps = xbT_slice^T @ w2v + ones x bv2
        wv = w2_sb["wv"]
        for j in range(NJB):
            vps = vps_p.tile([CP, D2], f32)
            for ct in range(NCT):
                nc.tensor.matmul(
                    vps[:],
                    lhsT=xbT[:, ct, ts(j, JB)],
                    rhs=wv[:, ct, :],
                    start=(ct == 0),
                    stop=False,
                )
            nc.tensor.matmul(
                vps[:], lhsT=ones1[:, 0:JB], rhs=bv2[:], start=False, stop=True
            )
            # scatter [j, (h d)] -> vaug[:, j, h*(D+1) : h*(D+1)+D]
            nc.scalar.activation(
                vaug[:, j, :].rearrange("p (h e) -> p h e", h=HPC)[:, :, 0:D],
                vps[:].rearrange("p (h d) -> p h d", h=HPC),
                AF.Copy,
            )

    if os.environ.get("KERNEL_PHASES") == "F":
        nc.gpsimd.dma_start(pT_d.rearrange("(a p) s -> a p s", p=CP)[0], qT[:])
        nc.gpsimd.dma_start(pT_d.rearrange("(a p) s -> a p s", p=CP)[1], kT[:])
        for t in range(8):
            nc.gpsimd.dma_start(
                pT_d.rearrange("(a p) s -> a p s", p=CP)[2][:, t * 130 : t * 130 + 130],
                vaug[:, t, :],
            )
        return

    # ---- P3: attention + interleaved P4 (wo projection of finished chunks) ----
    oT = oT_p.tile([D2, S], bf16)
    pT_v = pT_d.rearrange("(t p) s -> t p s", p=CP)
    with tc.tile_pool(name="sc_ps", bufs=2, space="PSUM") as sps, \
         tc.tile_pool(name="o_ps", bufs=1, space="PSUM") as ops, \
         tc.tile_pool(name="ex_sb", bufs=4) as exp_p, \
         tc.tile_pool(name="nrm_sb", bufs=4) as nrm_p, \
         tc.tile_pool(name="wo_sb2", bufs=3) as wsb:

        def wo_chunk(sc):
            # partial^T for s-chunk sc: 4 cc x 2 n2 matmuls from oT
            for cc in range(NCT):
                for n2 in range(SCHUNK // 512):
                    wps = ops.tile([CP, 512], f32, tag=f"o{cc % 2}",
                                   name=f"wops_{sc}_{cc}_{n2}")
                    nc.tensor.matmul(
                        wps[:],
                        lhsT=wo_sb[:, ts(cc, CP)],
                        rhs=oT[:, sc * SCHUNK + n2 * 512 : sc * SCHUNK + (n2 + 1) * 512],
                        start=True,
                        stop=True,
                    )
                    ot = wsb.tile([CP, 512], bf16)
                    nc.vector.tensor_copy(ot[:], wps[:])
                    nc.sync.dma_start(
                        pT_v[cc][:, sc * SCHUNK + n2 * 512 : sc * SCHUNK + (n2 + 1) * 512],
                        ot[:],
                    )

        for sc in range(NSC):
            o_ps = [
                ops.tile([D + 1, SCHUNK], f32, tag=f"o{h}", name=f"ops_{sc}_{h}")
                for h in range(HPC)
            ]
            for j in range(NJB):
                for h in range(HPC):
                    ps = sps.tile([JB, SCHUNK], f32)
                    for n2 in range(SCHUNK // 512):
                        nc.tensor.matmul(
                            ps[:, ts(n2, 512)],
                            lhsT=kT[h * D : (h + 1) * D, ts(j, JB)],
                            rhs=qT[h * D : (h + 1) * D, sc * SCHUNK + n2 * 512 : sc * SCHUNK + (n2 + 1) * 512],
                            start=True,
                            stop=True,
                        )
                    ex = exp_p.tile([JB, SCHUNK], bf16)
                    nc.scalar.activation(ex[:], ps[:], AF.Exp)
                    for n2 in range(SCHUNK // 512):
                        nc.tensor.matmul(
                            o_ps[h][:, ts(n2, 512)],
                            lhsT=vaug[:, j, h * (D + 1) : (h + 1) * (D + 1)],
                            rhs=ex[:, ts(n2, 512)],
                            start=(j == 0),
                            stop=(j == NJB - 1),
                        )
            for h in range(HPC):
                lnr = nrm_p.tile([1, SCHUNK], f32, tag="lnr")
                nc.scalar.activation(lnr[:], o_ps[h][D : D + 1, :], AF.Ln)
                rec = nrm_p.tile([1, SCHUNK], f32, tag="rec")
                nc.scalar.activation(rec[:], lnr[:], AF.Exp, scale=-1.0)
                o_f = nrm_p.tile([D, SCHUNK], f32, tag="of")
                nc.vector.tensor_copy(o_f[:], o_ps[h][0:D, :])
                bc = ops.tile([D, SCHUNK], f32, tag=f"o{h}", name=f"bc_{sc}_{h}")
                for n2 in range(SCHUNK // 512):
                    nc.tensor.matmul(
                        bc[:, ts(n2, 512)],
                        lhsT=ones64[:],
                        rhs=rec[:, ts(n2, 512)],
                        start=True,
                        stop=True,
                    )
                nc.vector.tensor_tensor(
                    oT[h * D : (h + 1) * D, ts(sc, SCHUNK)], o_f[:], bc[:], op=OP.mult
                )
            wo_chunk(sc)


_CACHE = {}


def _build():
    if "nc" in _CACHE:
        return _CACHE["nc"]
    import contextlib

    nc = bacc.Bacc("TRN2", target_bir_lowering=False, debug=False, enable_asserts=False)
    with tile.TileContext(nc) as tc:
        with contextlib.ExitStack() as ctx:
            _body(ctx, tc)
    nc.compile()
    _CACHE["nc"] = nc
    return nc


def _in_maps(inputs):
    x = np.ascontiguousarray(np.asarray(inputs["hidden_states"], dtype=np.float32))
    selg = (np.arange(CP)[:, None] // CPG == np.arange(GPT)[None, :]).astype(np.float32)
    selb = np.ascontiguousarray(selg.T)
    id128 = np.eye(CP, dtype=np.float32)
    bf = mybir.dt.np(bf16)
    ones64 = np.ones((1, D), dtype=np.float32)
    ones1 = np.ones((1, CP), dtype=np.float32).astype(bf)
    maps = []
    for c in range(N_CORES):
        b = c // (N_CORES // B)
        p = c % (N_CORES // B)
        sl = slice(p * D2, (p + 1) * D2)
        maps.append(
            {
                "x": x[b],
                "wq": np.ascontiguousarray(np.asarray(inputs["wq"], np.float32)[:, sl]),
                "wk": np.ascontiguousarray(np.asarray(inputs["wk"], np.float32)[:, sl]),
                "wv": np.ascontiguousarray(np.asarray(inputs["wv"], np.float32)[:, sl]),
                "wo": np.ascontiguousarray(np.asarray(inputs["wo"], np.float32)[sl, :]),
                "bq": np.ascontiguousarray(np.asarray(inputs["bq"], np.float32)[sl, None]),
                "bk": np.ascontiguousarray(np.asarray(inputs["bk"], np.float32)[sl, None]),
                "bv": np.ascontiguousarray(np.asarray(inputs["bv"], np.float32)[None, sl]),
                "gnw": np.asarray(inputs["gn_w"], np.float32),
                "gnb": np.asarray(inputs["gn_b"], np.float32),
                "selg": selg,
                "selb": selb,
                "id128": id128,
                "ones64": ones64,
                "ones1": ones1,
            }
        )
    return maps


def _assemble(inputs, results):
    x = np.asarray(inputs["hidden_states"], dtype=np.float32)
    bo = np.asarray(inputs["bo"], dtype=np.float32)
    out = np.zeros((B, S, C), dtype=np.float32)
    for c in range(N_CORES):
        b = c // (N_CORES // B)
        out[b] += results[c]["pT"].astype(np.float32).T
    out += bo
    out += x
    return out


def kernel(**inputs):
    nc = _build()
    maps = _in_maps(inputs)
    res = run_bass_kernel_spmd(nc, maps, list(range(N_CORES)))
    return _assemble(inputs, res.results)


if __name__ == "__main__":
    nc = _build()
    print("built ok")
